# revision 11
# baseline (speedup 1.0000x reference)
"""CARAFE content-aware upsampling as a hand-written Bass/Tile kernel for 8
Trainium2 NeuronCores.

Problem (hardcoded): X [4,256,128,128] f32, comp_w [64,256,1,1], comp_b [64],
enc_w [100,64,3,3], enc_b [100]  ->  out [4,256,256,256] f32.

Sharding: core = 2*b + h handles image b, output row-half h (128 of 256 rows).

Per-core pipeline (bf16 hot path, fp32 accumulation in PSUM):
  1x1 conv (PE) -> 3x3 conv (PE) -> exp (ScalarE) -> 5 x-shifted PE
  transposes of the softmax logits; bilinear via horizontal blend (x free),
  PE transpose to x-in-partitions, vertical blend; 25-tap reassembly as
  per-partition-scalar multiplies (DVE/ACT/GPSIMD) + per-shift-group sums,
  with the x-shift + cross-group sum done by matmuls against static shift
  matrices accumulating in PSUM; normalization folded into the PSUM copy;
  PE transpose back to channel partitions; DMA out.
"""
import numpy as np

B, C, H, W = 4, 256, 128, 128
C_MID, KC = 64, 100
S, K_UP = 2, 5
H2, W2 = H * S, W * S
N_CORES = 8

_cached = None


def _build_nc():
    import concourse.bass as bass
    import concourse.tile as tile
    from concourse import bacc, mybir

    f32 = mybir.dt.float32
    b16 = mybir.dt.bfloat16
    AL = mybir.AluOpType
    AF = mybir.ActivationFunctionType

    nc = bacc.Bacc(None, target_bir_lowering=False)

    slab_d = nc.dram_tensor("slab", [2, 128, 70, 130], f32, kind="ExternalInput")
    halos_d = nc.dram_tensor("halos", [2, 128, 2, 128], f32, kind="ExternalInput")
    cwT_d = nc.dram_tensor("cwT", [2, 128, C_MID], f32, kind="ExternalInput")
    cb_d = nc.dram_tensor("cb", [C_MID, 1], f32, kind="ExternalInput")
    ewT_d = nc.dram_tensor("ewT", [9, C_MID, KC], f32, kind="ExternalInput")
    eb_d = nc.dram_tensor("eb", [KC, 1], f32, kind="ExternalInput")
    eye_d = nc.dram_tensor("eye", [128, 128], b16, kind="ExternalInput")
    emat_d = nc.dram_tensor("emat", [5, 128, 128], b16, kind="ExternalInput")
    es_d = nc.dram_tensor("es", [128, 2], f32, kind="ExternalInput")
    out_d = nc.dram_tensor("out", [2, 128, 128, 256], f32, kind="ExternalOutput")

    from contextlib import ExitStack
    with tile.TileContext(nc) as tc:
        with (
            tc.tile_pool(name="const", bufs=1) as const,
            tc.tile_pool(name="big", bufs=1) as big,
            tc.tile_pool(name="rowio", bufs=2) as rowio,
            tc.tile_pool(name="t13", bufs=36) as t13p,
            tc.tile_pool(name="xur", bufs=30) as xurp,
            tc.tile_pool(name="prod", bufs=4) as prodp,
            tc.tile_pool(name="xh", bufs=6) as xhp,
            tc.tile_pool(name="acc", bufs=3) as accp,
            tc.tile_pool(name="stg", bufs=6) as stgp,
            tc.tile_pool(name="dram", bufs=1, space="DRAM") as dram,
        ):
            ph1 = ExitStack()
            wcps = ph1.enter_context(tc.tile_pool(name="wcps", bufs=2, space="PSUM"))
            weps = ph1.enter_context(tc.tile_pool(name="weps", bufs=2, space="PSUM"))
            wtps = ph1.enter_context(tc.tile_pool(name="wtps", bufs=2, space="PSUM"))
            wrow = ph1.enter_context(tc.tile_pool(name="wrow", bufs=3))
            # ---------- constants ----------
            eye = const.tile([128, 128], b16)
            nc.sync.dma_start(eye[:], eye_d[:])
            emat = const.tile([128, 5, 128], b16)
            nc.sync.dma_start(emat[:], emat_d.rearrange("j p m -> p j m"))
            es = const.tile([128, 2], f32)
            nc.sync.dma_start(es[:], es_d[:])
            cb = const.tile([C_MID, 1], f32)
            nc.sync.dma_start(cb[:], cb_d[:])
            eb = const.tile([KC, 1], f32)
            nc.sync.dma_start(eb[:], eb_d[:])
            cwTf = const.tile([128, 2, C_MID], f32)
            nc.sync.dma_start(cwTf[:], cwT_d.rearrange("a b c -> b a c"))
            cwT = const.tile([128, 2, C_MID], b16)
            nc.vector.tensor_copy(cwT[:], cwTf[:])
            ewTf = const.tile([C_MID, 9, KC], f32)
            nc.sync.dma_start(ewTf[:], ewT_d.rearrange("n k m -> k n m"))
            ewT = const.tile([C_MID, 9, KC], b16)
            nc.vector.tensor_copy(ewT[:], ewTf[:])

            # ---------- load X slab, cast to bf16 (resident) ----------
            sbf = big.tile([128, 2, 70, 130], b16)
            for ch in range(2):
                for rs in range(0, 70, 10):
                    sf = rowio.tile([128, 10, 130], f32, tag="sfin")
                    nc.sync.dma_start(sf[:], slab_d[ch, :, rs:rs + 10, :])
                    nc.vector.tensor_copy(sbf[:, ch, rs:rs + 10, :], sf[:])
            hf = rowio.tile([128, 2, 2, 128], f32, tag="hfin")
            nc.sync.dma_start(hf[:], halos_d.rearrange("a b c d -> b a c d"))
            hbf = const.tile([128, 2, 2, 128], b16)
            nc.vector.tensor_copy(hbf[:], hf[:])

            # ---------- phase 1a: 1x1 conv -> WcPad [64, 66, 130] ----------
            wcpad = big.tile([C_MID, 66, 130], b16)
            nc.vector.memset(wcpad[:, :, 0:1], 0.0)
            nc.vector.memset(wcpad[:, :, 129:130], 0.0)
            for t in range(66):
                ps = wcps.tile([C_MID, 128], f32)
                for ch in range(2):
                    if t == 0:
                        rhs = hbf[:, ch, 0, :]
                    elif t == 65:
                        rhs = hbf[:, ch, 1, :]
                    else:
                        rhs = sbf[:, ch, t + 2, 1:129]
                    nc.tensor.matmul(ps[:], cwT[:, ch, :], rhs,
                                     start=(ch == 0), stop=(ch == 1))
                nc.scalar.activation(wcpad[:, t, 1:129], ps[:],
                                     AF.Identity, bias=cb[:], scale=1.0)

            # ---------- phase 1b: 3x3 conv, exp, shifted transposes ----------
            wshd = dram.tile([64, 128, 5, KC], b16)   # [y'][x'][j][kc]
            recip = big.tile([128, 64, 4], f32)
            for yp in range(64):
                we = weps.tile([KC, 128], f32)
                for dy in range(3):
                    for dx in range(3):
                        nc.tensor.matmul(we[:], ewT[:, 3 * dy + dx, :],
                                         wcpad[:, yp + dy, dx:dx + 128],
                                         start=(dy == 0 and dx == 0),
                                         stop=(dy == 2 and dx == 2))
                wep = wrow.tile([KC, 132], b16, tag="wep")
                nc.vector.memset(wep[:, 0:2], 0.0)
                nc.vector.memset(wep[:, 130:132], 0.0)
                nc.scalar.activation(wep[:, 2:130], we[:], AF.Exp,
                                     bias=eb[:], scale=1.0)
                wrt = wrow.tile([128, 5, KC], b16, tag="wrt")
                for j in range(5):
                    pw = wtps.tile([128, KC], b16)
                    nc.tensor.transpose(pw[:], wep[:, (4 - j):(4 - j) + 128],
                                        eye[0:KC, 0:KC])
                    if j % 2 == 0:
                        nc.vector.tensor_copy(wrt[:, j, :], pw[:])
                    else:
                        nc.scalar.copy(wrt[:, j, :], pw[:])
                nc.sync.dma_start(wshd[yp], wrt[:])
                den = wrow.tile([128, 4], f32, tag="den")
                nc.vector.tensor_reduce(
                    out=den[:],
                    in_=wrt[:, 2, :].rearrange("x (k sub) -> x sub k", sub=4),
                    axis=mybir.AxisListType.X, op=AL.add)
                nc.vector.reciprocal(recip[:, yp, :], den[:])

            # ---------- phase 2: features + reassembly, row-streamed ----------
            ph1.close()
            ph2 = ExitStack()
            xhtps = ph2.enter_context(tc.tile_pool(name="xhtps", bufs=2, space="PSUM"))
            opsum = ph2.enter_context(tc.tile_pool(name="ops", bufs=2, space="PSUM"))
            otps = ph2.enter_context(tc.tile_pool(name="otps", bufs=3, space="PSUM"))
            t1 = {}
            t3 = {}
            xur = {}

            def make_xht_row(trow):
                tarow = xhp.tile([128, 2, 130], b16, tag="tarow")
                tbrow = xhp.tile([128, 2, 130], b16, tag="tbrow")
                nc.vector.tensor_scalar(tarow[:], sbf[:, :, trow, :], 0.25,
                                        None, AL.mult)
                nc.scalar.mul(tbrow[:], sbf[:, :, trow, :], 0.75)
                for s in range(2):
                    xh = xhp.tile([128, 2, 128], b16, tag="xh")
                    for ch in range(2):
                        if s == 0:
                            a0 = tarow[:, ch, 0:128]
                            a1 = tbrow[:, ch, 1:129]
                        else:
                            a0 = tbrow[:, ch, 1:129]
                            a1 = tarow[:, ch, 2:130]
                        nc.vector.tensor_tensor(xh[:, ch, :], a0, a1, AL.add)
                    pt = xhtps.tile([128, 2, 128], b16)
                    for ch in range(2):
                        nc.tensor.transpose(pt[:, ch, :], xh[:, ch, :], eye[:])
                    r1 = t13p.tile([128, 256], b16, tag="t1")
                    r3 = t13p.tile([128, 256], b16, tag="t3")
                    nc.vector.tensor_scalar(r1[:], pt[:], 0.25, None, AL.mult)
                    nc.scalar.mul(r3[:], pt[:], 0.75)
                    t1[(s, trow)] = r1
                    t3[(s, trow)] = r3

            def make_xur_row(r, s, up):
                xt = xurp.tile([128, 256], b16, tag="xur")
                if r == 0:
                    nc.vector.tensor_tensor(xt[:], t1[(s, up)][:],
                                            t3[(s, up + 1)][:], AL.add)
                else:
                    nc.vector.tensor_tensor(xt[:], t3[(s, up + 1)][:],
                                            t1[(s, up + 2)][:], AL.add)
                if up in (0, 1):
                    nc.vector.tensor_scalar(xt[:], xt[:], es[:, 0:1], None, AL.mult)
                elif up in (66, 67):
                    nc.vector.tensor_scalar(xt[:], xt[:], es[:, 1:2], None, AL.mult)
                xur[(r, s, up)] = xt

            def mul_engine(j, i):
                k = (5 * i + j) % 5
                if k in (0, 1):
                    return "v"
                if k in (2, 3):
                    return "s"
                return "g"

            for trow in range(9):
                make_xht_row(trow)
            for r in range(2):
                for s in range(2):
                    for up in range(5):
                        make_xur_row(r, s, up)

            def emit_group(yp, r, s, j, po, wshf):
                sub = 2 * r + s
                pr = prodp.tile([128, 5, 256], b16, tag="pr")
                for i in range(5):
                    kc = 4 * (5 * i + j) + sub
                    sc = wshf[:, j, kc:kc + 1]
                    src = xur[(r, s, yp + i)][:]
                    eng = mul_engine(j, i)
                    if eng == "v":
                        nc.vector.tensor_scalar(pr[:, i, :], src, sc, None, AL.mult)
                    elif eng == "g":
                        nc.gpsimd.tensor_scalar(pr[:, i, :], src, sc, None, AL.mult)
                    else:
                        nc.scalar.mul(pr[:, i, :], src, sc)
                l1 = prodp.tile([128, 2, 256], b16, tag="l1")
                nc.vector.tensor_tensor(l1[:], pr[:, 0:4:2, :], pr[:, 1:4:2, :],
                                        AL.add)
                l2 = prodp.tile([128, 256], b16, tag="l2")
                nc.gpsimd.tensor_tensor(l2[:], l1[:, 0, :], l1[:, 1, :], AL.add)
                sj = prodp.tile([128, 256], b16, tag="sj")
                nc.vector.tensor_tensor(sj[:], l2[:], pr[:, 4, :], AL.add)
                nc.tensor.matmul(po[:], emat[:, j, :], sj[:],
                                 start=(j == 0), stop=(j == 4))

            def emit_sub(yp, r, s, stg, wshf):
                sub = 2 * r + s
                po = opsum.tile([128, 256], f32)
                for j in range(5):
                    emit_group(yp, r, s, j, po, wshf)
                acc = accp.tile([128, 256], b16)
                nc.scalar.mul(acc[:], po[:], recip[:, yp, sub:sub + 1])
                for ch in range(2):
                    ot = otps.tile([128, 128], b16)
                    nc.tensor.transpose(ot[:], acc[:, ch * 128:(ch + 1) * 128],
                                        eye[:])
                    if s == 0:
                        nc.vector.tensor_copy(stg[ch][:, :, 0], ot[:])
                    else:
                        nc.scalar.copy(stg[ch][:, :, 1], ot[:])

            def emit_row(yp, r, wshf):
                Y = 2 * yp + r
                stg0 = stgp.tile([128, 128, 2], f32, tag="stg0")
                stg1 = stgp.tile([128, 128, 2], f32, tag="stg1")
                stg = [stg0, stg1]
                for s in range(2):
                    emit_sub(yp, r, s, stg, wshf)
                for ch in range(2):
                    nc.sync.dma_start(out_d[ch, :, Y, :],
                                      stg[ch].rearrange("p a b -> p (a b)"))

            for yp in range(64):
                if yp > 0:
                    need_t = yp + 8
                    if need_t < 70:
                        make_xht_row(need_t)
                    upn = yp + 4
                    if upn < 68:
                        for r in range(2):
                            for s in range(2):
                                make_xur_row(r, s, upn)
                wshf = accp.tile([128, 5, KC], f32, tag="wshf")
                nc.gpsimd.dma_start(wshf[:], wshd[yp])
                for r in range(2):
                    emit_row(yp, r, wshf)
                for s in range(2):
                    t1.pop((s, yp), None)
                    t3.pop((s, yp), None)
                for r in range(2):
                    for s in range(2):
                        xur.pop((r, s, yp), None)
            ph2.close()

    nc.compile()
    return nc


def _host_prep(X, comp_w, comp_b, enc_w, enc_b):
    import ml_dtypes
    comp_wT = np.ascontiguousarray(comp_w[:, :, 0, 0].T)        # [256, 64]
    enc_wT = np.ascontiguousarray(
        enc_w.transpose(2, 3, 1, 0).reshape(9, C_MID, KC))      # [9, 64, 100]
    eye16 = np.eye(128, dtype=ml_dtypes.bfloat16)
    emat = np.zeros((5, 128, 128), np.float32)
    for j in range(5):
        dj = j - 2
        xs = np.arange(128) + dj
        m = (xs >= 0) & (xs < 128)
        emat[j, xs[m], np.arange(128)[m]] = 1.0
    emat16 = emat.astype(ml_dtypes.bfloat16)
    cores = []
    for core in range(N_CORES):
        b, h = core // 2, core % 2
        r0 = 64 * h
        Xb = X[b]
        rows = np.clip(np.arange(r0 - 3, r0 + 67), 0, H - 1)
        slab = Xb[:, rows, :]
        slab = np.concatenate([slab[:, :, :1], slab, slab[:, :, -1:]], axis=2)
        slab = np.ascontiguousarray(slab.reshape(2, 128, 70, 130))
        top = np.zeros((C, W), np.float32) if h == 0 else Xb[:, r0 - 1]
        bot = Xb[:, 64] if h == 0 else np.zeros((C, W), np.float32)
        halos = np.ascontiguousarray(
            np.stack([top, bot], axis=1).reshape(2, 128, 2, 128))
        es = np.zeros((128, 2), np.float32)
        es[:, 0] = 0.0 if h == 0 else 1.0
        es[:, 1] = 1.0 if h == 0 else 0.0
        cores.append({
            "slab": slab, "halos": halos,
            "cwT": np.ascontiguousarray(comp_wT.reshape(2, 128, C_MID)),
            "cb": comp_b.reshape(C_MID, 1).astype(np.float32),
            "ewT": enc_wT.astype(np.float32),
            "eb": enc_b.reshape(KC, 1).astype(np.float32),
            "eye": eye16, "emat": emat16, "es": es,
        })
    return cores


_runner = None


def _build_runner(nc):
    """Build a cached multi-core runner (the same PJRT path that
    bass_utils.run_bass_kernel_spmd takes under axon, but with the jitted
    executable cached across calls so the NEFF is loaded once)."""
    import jax
    import numpy as _np
    from jax.sharding import Mesh, PartitionSpec
    from jax.experimental.shard_map import shard_map
    from concourse import mybir
    from concourse import bass2jax
    from concourse.bass2jax import _bass_exec_p, install_neuronx_cc_hook, \
        partition_id_tensor

    install_neuronx_cc_hook()
    partition_name = (nc.partition_id_tensor.name
                      if nc.partition_id_tensor else None)
    in_names, out_names, out_avals, zero_outs = [], [], [], []
    for alloc in nc.m.functions[0].allocations:
        if not isinstance(alloc, mybir.MemoryLocationSet):
            continue
        name = alloc.memorylocations[0].name
        if alloc.kind == "ExternalInput":
            if name != partition_name:
                in_names.append(name)
        elif alloc.kind == "ExternalOutput":
            out_names.append(name)
            shape = tuple(alloc.tensor_shape)
            dtype = mybir.dt.np(alloc.dtype)
            out_avals.append(jax.core.ShapedArray(shape, dtype))
            zero_outs.append(_np.zeros(shape, dtype))
    n_params = len(in_names)
    n_outs = len(out_avals)
    all_in_names = list(in_names) + list(out_names)
    if partition_name is not None:
        all_in_names.append(partition_name)
    donate = tuple(range(n_params, n_params + n_outs))

    def _body(*args):
        operands = list(args)
        if partition_name is not None:
            operands.append(partition_id_tensor())
        outs = _bass_exec_p.bind(
            *operands,
            out_avals=tuple(out_avals),
            in_names=tuple(all_in_names),
            out_names=tuple(out_names),
            lowering_input_output_aliases=(),
            sim_require_finite=True,
            sim_require_nnan=True,
            nc=nc,
        )
        return tuple(outs)

    devices = jax.devices()[:N_CORES]
    mesh = Mesh(_np.asarray(devices), ("core",))
    in_specs = (PartitionSpec("core"),) * (n_params + n_outs)
    out_specs = (PartitionSpec("core"),) * len(out_names)
    sharded = jax.jit(
        shard_map(_body, mesh=mesh, in_specs=in_specs, out_specs=out_specs,
                  check_rep=False),
        donate_argnums=donate, keep_unused=True)

    def run(in_maps):
        per_core = [[_np.asarray(m[name]) for name in in_names]
                    for m in in_maps]
        concat_in = [
            _np.concatenate([per_core[c][i] for c in range(N_CORES)], axis=0)
            for i in range(n_params)
        ]
        concat_zeros = [
            _np.zeros((N_CORES * z.shape[0], *z.shape[1:]), z.dtype)
            for z in zero_outs
        ]
        out_arrs = sharded(*concat_in, *concat_zeros)
        return [
            {name: _np.asarray(out_arrs[i]).reshape(
                N_CORES, *out_avals[i].shape)[c]
             for i, name in enumerate(out_names)}
            for c in range(N_CORES)
        ]

    return run


def kernel(X, comp_w, comp_b, enc_w, enc_b):
    global _cached, _runner
    import sys
    if "/opt/trn_rl_repo" not in sys.path:
        sys.path.insert(0, "/opt/trn_rl_repo")

    X = np.asarray(X, np.float32)
    comp_w = np.asarray(comp_w, np.float32)
    comp_b = np.asarray(comp_b, np.float32)
    enc_w = np.asarray(enc_w, np.float32)
    enc_b = np.asarray(enc_b, np.float32)

    if _cached is None:
        _cached = _build_nc()
    if _runner is None:
        _runner = _build_runner(_cached)

    in_maps = _host_prep(X, comp_w, comp_b, enc_w, enc_b)
    results = _runner(in_maps)
    out = np.empty((B, C, H2, W2), np.float32)
    for core in range(N_CORES):
        b, h = core // 2, core % 2
        o = np.asarray(results[core]["out"])  # [2, 128, 128, 256]
        out[b, 0:128, h * 128:(h + 1) * 128, :] = o[0]
        out[b, 128:256, h * 128:(h + 1) * 128, :] = o[1]
    return out


def device_exec_time_ns(inputs):
    """Time the on-device execution with device-resident inputs (min of 3)."""
    import time
    import jax
    global _cached
    import sys as _sys
    if "/opt/trn_rl_repo" not in _sys.path:
        _sys.path.insert(0, "/opt/trn_rl_repo")
    if _cached is None:
        _cached = _build_nc()
    nc = _cached
    import numpy as _np
    from jax.sharding import Mesh, PartitionSpec, NamedSharding
    from jax.experimental.shard_map import shard_map
    from concourse import mybir
    from concourse.bass2jax import _bass_exec_p, install_neuronx_cc_hook, \
        partition_id_tensor

    install_neuronx_cc_hook()
    partition_name = (nc.partition_id_tensor.name
                      if nc.partition_id_tensor else None)
    in_names, out_names, out_avals = [], [], []
    for alloc in nc.m.functions[0].allocations:
        if not isinstance(alloc, mybir.MemoryLocationSet):
            continue
        name = alloc.memorylocations[0].name
        if alloc.kind == "ExternalInput":
            if name != partition_name:
                in_names.append(name)
        elif alloc.kind == "ExternalOutput":
            out_names.append(name)
            out_avals.append(jax.core.ShapedArray(
                tuple(alloc.tensor_shape), mybir.dt.np(alloc.dtype)))
    n_params = len(in_names)
    all_in_names = list(in_names) + list(out_names)
    if partition_name is not None:
        all_in_names.append(partition_name)

    def _body(*args):
        operands = list(args)
        if partition_name is not None:
            operands.append(partition_id_tensor())
        return tuple(_bass_exec_p.bind(
            *operands,
            out_avals=tuple(out_avals),
            in_names=tuple(all_in_names),
            out_names=tuple(out_names),
            lowering_input_output_aliases=(),
            sim_require_finite=True,
            sim_require_nnan=True,
            nc=nc))

    devices = jax.devices()[:N_CORES]
    mesh = Mesh(_np.asarray(devices), ("core",))
    nin = n_params + len(out_names)
    fn = jax.jit(shard_map(_body, mesh=mesh,
                           in_specs=(PartitionSpec("core"),) * nin,
                           out_specs=(PartitionSpec("core"),) * len(out_names),
                           check_rep=False))
    in_maps = _host_prep(inputs["X"], inputs["comp_w"], inputs["comp_b"],
                         inputs["enc_w"], inputs["enc_b"])
    sh = NamedSharding(mesh, PartitionSpec("core"))
    args = []
    for i, name in enumerate(in_names):
        cat = _np.concatenate([_np.asarray(in_maps[c][name])
                               for c in range(N_CORES)], axis=0)
        args.append(jax.device_put(cat, sh))
    for av in out_avals:
        args.append(jax.device_put(
            _np.zeros((N_CORES * av.shape[0], *av.shape[1:]), av.dtype), sh))
    r = fn(*args)
    jax.block_until_ready(r)
    r = fn(*args)
    jax.block_until_ready(r)
    best = None
    for _ in range(6):
        t0 = time.time()
        r = fn(*args)
        jax.block_until_ready(r)
        dt = time.time() - t0
        best = dt if best is None or dt < best else best
    return int(best * 1e9)


if __name__ == "__main__":
    d = np.load("/tmp/inputs.npz")
    out = kernel(**{k: d[k] for k in ("X", "comp_w", "comp_b", "enc_w", "enc_b")})
    exp = np.load("/tmp/expected.npy")
    err = np.abs(out - exp)
    print("absmax", err.max(), "rel", err.max() / np.abs(exp).max())
